# revision 6
# baseline (speedup 1.0000x reference)
"""Attention-pooling kernel for TRN2 (8 NeuronCores, batch-sharded), fp16.

Computes, for h[B,T,D], W_w[A,D], b_w[A], u_w[A]:
    u     = tanh(h @ W_w.T + b_w)          [B,T,A]
    score = u @ u_w                        [B,T]
    alpha = softmax(score, axis=T)
    s     = einsum('bt,btd->bd', alpha, h) [B,D]

Key design vs the fp32 baseline: h is downconverted to fp16 on the host,
HALVING the HBM stream (8 MiB/core, ~23.3us at the 360 GB/s model rate).
All matmul paths run 16-bit (transposes, u-matmul, pooling); the tanh/exp
and score dot stay fp32. The softmax shift is a per-example max (computed
on-device via DVE free-dim max + GPSIMD cross-partition max) so the e
weights fit fp16 for the pooling matmul; the shift cancels in the host
normalization e/sum(e).

Pipeline: 512-token chunks, 32 per core. Per chunk the PE does
8 transposes + 2 u-matmuls + 4 score dots + 16 pooling dots (~0.9us),
DVE drains the transposed PSUM tile to SBUF (~0.8us), ACT does
tanh (+ per-example exp) (~0.9us), DMA streams the next chunk (~0.73us).
PE is the roofline engine.
"""

import numpy as np

import concourse.bacc as bacc
import concourse.bass as bass
import concourse.bass_isa as bass_isa
import concourse.mybir as mybir
import concourse.tile as tile
from concourse.bass_utils import run_bass_kernel_spmd

B, T, D, A = 32, 4096, 256, 128
NCORES = 8
BPC = B // NCORES          # examples per core
CHUNK = 512                # tokens per processing chunk
NSUB = CHUNK // 128        # 128-token subchunks per chunk (4)
NCHUNK = T // CHUNK        # chunks per example (8)
NCH = BPC * NCHUNK         # chunks per core (32)
ECOLS = T // 128           # e columns per example (32)

F32 = mybir.dt.float32
F16 = mybir.dt.float16

LAG_U = 2                  # u-matmul trails transpose (DVE copy latency)
LAG_S = 3                  # score trails transpose (tanh latency)
LAG_P = 12                 # pool(b) at iteration 8*b + LAG_P


def build_nc():
    nc = bacc.Bacc(
        "TRN2",
        target_bir_lowering=False,
        debug=False,
        num_devices=NCORES,
    )

    h_d = nc.dram_tensor("h16", [BPC, T, D], F16, kind="ExternalInput").ap()
    # consts [128, 130] f32: [wt0_16(64) | wt1_16(64) | bw | uw]
    c_d = nc.dram_tensor("consts", [128, 130], F32, kind="ExternalInput").ap()
    s_d = nc.dram_tensor("s", [128, 2 * BPC], F32, kind="ExternalOutput").ap()
    e_d = nc.dram_tensor("e16", [128, ECOLS * BPC], F16,
                         kind="ExternalOutput").ap()

    with tile.TileContext(nc) as tc:
        with (
            tc.tile_pool(name="const", bufs=1) as const_pool,
            tc.tile_pool(name="hall", bufs=1) as h_pool,
            tc.tile_pool(name="hT", bufs=4) as hT_pool,
            tc.tile_pool(name="u", bufs=4) as u_pool,
            tc.tile_pool(name="out", bufs=1) as out_pool,
            tc.tile_pool(name="mx", bufs=2) as mx_pool,
            tc.tile_pool(name="pt", bufs=3, space="PSUM") as pt_pool,
            tc.tile_pool(name="pu", bufs=2, space="PSUM") as pu_pool,
            tc.tile_pool(name="psT", bufs=2, space="PSUM") as psT_pool,
            tc.tile_pool(name="ps", bufs=1, space="PSUM") as ps_pool,
        ):
            h_all = h_pool.tile([128, NCH * NSUB * D], F16)

            def load_chunk(k, pieces=1):
                b, c = divmod(k, NCHUNK)
                src = h_d[b, c * CHUNK:(c + 1) * CHUNK, :] \
                    .rearrange("(p n) d -> p n d", n=NSUB)
                step = NSUB // pieces
                for q in range(pieces):
                    nc.sync.dma_start(
                        out=h_all[:, k * NSUB * D + q * step * D:
                                  k * NSUB * D + (q + 1) * step * D]
                        .rearrange("p (n d) -> p n d", d=D),
                        in_=src[:, q * step:(q + 1) * step, :],
                    )

            # identity built on-device (DVE) so the first transposes are
            # gated only by the first h piece, not a consts DMA
            ident_t = const_pool.tile([128, 128], F16)
            nc.vector.memset(ident_t[:], 1.0)
            nc.gpsimd.affine_select(
                ident_t[:], ident_t[:], pattern=[[-1, 128]],
                compare_op=mybir.AluOpType.is_equal, fill=0.0,
                base=0, channel_multiplier=1)
            ident = ident_t[:]
            # first h chunk owns the first DGE slots; consts follow
            load_chunk(0, pieces=2)
            const_sb = const_pool.tile([128, 130], F32)
            nc.sync.dma_start(out=const_sb[:], in_=c_d[:])
            wt = [const_sb[:, 0:64].bitcast(F16),         # [128, 128] each
                  const_sb[:, 64:128].bitcast(F16)]
            bw_sb = const_sb[:, 128:129]
            uw_sb = const_sb[:, 129:130]
            load_chunk(1)

            s_out = out_pool.tile([128, 2 * BPC], F32)
            e_out = out_pool.tile([128, ECOLS * BPC], F16)

            hT_of = {}
            u_of = {}
            psT_of = {}
            negm_of = {}

            def stage_transpose(k):
                pt = pt_pool.tile([128, NSUB * 256], F16, tag="pt",
                                  name=f"pt_{k}")
                base = k * NSUB * D
                for kd in range(2):
                    for n in range(NSUB):
                        nc.tensor.matmul(
                            pt[:, kd * 512 + n * 128:kd * 512 + (n + 1) * 128],
                            h_all[:, base + n * D + kd * 128:
                                  base + n * D + (kd + 1) * 128],
                            ident,
                            is_transpose=True,
                            start=(kd == 0 and n == 0),
                            stop=(kd == 1 and n == NSUB - 1),
                        )
                hT = hT_pool.tile([128, NSUB * 256], F16, tag="hT",
                                  name=f"hT_{k}")
                nc.vector.tensor_copy(hT[:], pt[:])
                hT_of[k] = hT

            def stage_umm(k):
                hT = hT_of.pop(k)
                pu = pu_pool.tile([128, CHUNK], F32, tag="pu", name=f"pu_{k}")
                for j in range(2):
                    nc.tensor.matmul(
                        pu[:],
                        wt[j],
                        hT[:, j * 512:(j + 1) * 512],
                        start=(j == 0),
                        stop=(j == 1),
                    )
                u_sb = u_pool.tile([128, CHUNK], F32, tag="u", name=f"u_{k}")
                nc.scalar.activation(
                    u_sb[:], pu[:],
                    mybir.ActivationFunctionType.Tanh,
                    bias=bw_sb, scale=1.0,
                )
                u_of[k] = u_sb

            def stage_score(k):
                b, c = divmod(k, NCHUNK)
                u_sb = u_of.pop(k)
                if c == 0:
                    psT_of[b] = psT_pool.tile([128, ECOLS], F32, tag="psT",
                                              name=f"psT_{b}")
                psT = psT_of[b]
                for n in range(NSUB):
                    nc.tensor.matmul(
                        psT[:, c * NSUB + n:c * NSUB + n + 1],
                        u_sb[:, n * 128:(n + 1) * 128],
                        uw_sb,
                        start=(c == 0 and n == 0),
                        stop=(c == NCHUNK - 1 and n == NSUB - 1),
                    )

            def stage_maxexp(b):
                psT = psT_of.pop(b)
                mx = mx_pool.tile([128, 1], F32, tag="mx", name=f"mx_{b}")
                nc.vector.tensor_reduce(
                    mx[:], psT[:], axis=mybir.AxisListType.X,
                    op=mybir.AluOpType.max)
                mall = mx_pool.tile([128, 1], F32, tag="mall",
                                    name=f"mall_{b}")
                nc.gpsimd.partition_all_reduce(
                    mall[:], mx[:], channels=128,
                    reduce_op=bass_isa.ReduceOp.max)
                negm = mx_pool.tile([128, 1], F32, tag="negm",
                                    name=f"negm_{b}")
                nc.vector.tensor_reduce(
                    negm[:], mall[:], axis=mybir.AxisListType.X,
                    op=mybir.AluOpType.max, negate=True)
                nc.scalar.activation(
                    e_out[:, b * ECOLS:(b + 1) * ECOLS], psT[:],
                    mybir.ActivationFunctionType.Exp,
                    bias=negm[:, 0:1], scale=1.0,
                )
                nc.scalar.dma_start(
                    out=e_d[:, b * ECOLS:(b + 1) * ECOLS],
                    in_=e_out[:, b * ECOLS:(b + 1) * ECOLS])
                negm_of[b] = negm

            def stage_pool(b):
                ps = ps_pool.tile([128, 2], F32, tag="ps", name=f"ps_{b}")
                for c in range(NCHUNK):
                    base = (b * NCHUNK + c) * NSUB * D
                    for n in range(NSUB):
                        for kd in range(2):
                            nc.tensor.matmul(
                                ps[:, kd:kd + 1],
                                h_all[:, base + n * D + kd * 128:
                                      base + n * D + (kd + 1) * 128],
                                e_out[:, b * ECOLS + c * NSUB + n:
                                      b * ECOLS + c * NSUB + n + 1],
                                start=(c == 0 and n == 0 and kd == 0),
                                stop=(c == NCHUNK - 1 and n == NSUB - 1
                                      and kd == 1),
                            )
                nc.scalar.copy(s_out[:, 2 * b:2 * b + 2], ps[:, 0:2])
                nc.sync.dma_start(out=s_d[:, 2 * b:2 * b + 2],
                                  in_=s_out[:, 2 * b:2 * b + 2])

            NITER = 8 * (BPC - 1) + LAG_P + 1
            for i in range(NITER):
                if i + 2 < NCH:
                    load_chunk(i + 2)
                if 0 <= i - LAG_U < NCH:
                    stage_umm(i - LAG_U)
                if 0 <= i - LAG_S < NCH:
                    stage_score(i - LAG_S)
                    if (i - LAG_S) % NCHUNK == NCHUNK - 1:
                        stage_maxexp((i - LAG_S) // NCHUNK)
                if i >= LAG_P and (i - LAG_P) % NCHUNK == 0 \
                        and (i - LAG_P) // NCHUNK < BPC:
                    bb = (i - LAG_P) // NCHUNK
                    stage_pool(bb)
                if i < NCH:
                    stage_transpose(i)


    nc.compile()
    return nc


_NC_CACHE = {}


def _get_nc():
    if "nc" not in _NC_CACHE:
        _NC_CACHE["nc"] = build_nc()
    return _NC_CACHE["nc"]


def _pack16_pairs(x16):
    """[P, 2n] fp16 -> [P, n] f32 words with (even, odd) halves packed."""
    u = np.ascontiguousarray(x16).view(np.uint16)
    w = u[:, 0::2].astype(np.uint32) | (u[:, 1::2].astype(np.uint32) << 16)
    return np.ascontiguousarray(w).view(np.float32)


def _make_in_maps(h, W_w, b_w, u_w):
    h16 = np.ascontiguousarray(h, dtype=np.float32).astype(np.float16)
    W16 = np.ascontiguousarray(W_w, dtype=np.float32).astype(np.float16)
    wt0 = _pack16_pairs(np.ascontiguousarray(W16[:, 0:128].T))    # [128, 64]
    wt1 = _pack16_pairs(np.ascontiguousarray(W16[:, 128:256].T))  # [128, 64]
    consts = np.concatenate(
        [wt0, wt1,
         np.asarray(b_w, np.float32).reshape(A, 1),
         np.asarray(u_w, np.float32).reshape(A, 1)], axis=1)
    consts = np.ascontiguousarray(consts)
    return [
        {"h16": h16[i * BPC:(i + 1) * BPC], "consts": consts}
        for i in range(NCORES)
    ]


def _postprocess(s_raw, e_raw):
    """s_raw [128, 2*BPC] f32, e_raw [128, ECOLS*BPC] f16 -> s [BPC, D]."""
    s = np.empty((BPC, D), np.float64)
    e64 = np.asarray(e_raw, np.float16).astype(np.float64)
    for b in range(BPC):
        esum = e64[:, b * ECOLS:(b + 1) * ECOLS].sum()
        s[b, 0:128] = np.asarray(s_raw[:, 2 * b], np.float64) / esum
        s[b, 128:256] = np.asarray(s_raw[:, 2 * b + 1], np.float64) / esum
    return s.astype(np.float32)


def kernel(h, W_w, b_w, u_w):
    nc = _get_nc()
    in_maps = _make_in_maps(h, W_w, b_w, u_w)
    res = run_bass_kernel_spmd(nc, in_maps, core_ids=list(range(NCORES)))
    out = np.concatenate(
        [_postprocess(res.results[i]["s"], res.results[i]["e16"])
         for i in range(NCORES)], axis=0)
    return out.astype(np.float32)
